# revision 12
# baseline (speedup 1.0000x reference)
"""Self-contained Trainium2 kernel for nn_DynamicConv2D (moe_routing).

Contract: kernel(**inputs) takes FULL unsharded inputs (numpy), returns the
FULL output [32, 64, 64, 128] float32. Internally shards batch across 8
NeuronCores (4 samples each), runs a Bass/Tile kernel via
run_bass_kernel_spmd, and gathers.

Device-side work per sample:
  pool  = sum(x) over H,W            (piecewise partial reduces on ACT+DVE
                                      chasing the input DMA; 1/4096 folded
                                      into R on host)
  att   = softmax(relu(pool@R')@A')  (tiny PE matmuls + ACT relu/exp + DVE recip)
  wmix  = sum_k att[k] * bank[k]     (DVE scalar_tensor_tensor MACs, fp16,
                                      emitted in 3 tap-groups so the conv can
                                      start after the first group is mixed)
  conv  = 9-tap shifted fp16 matmuls accumulated in PSUM, per 512-pos chunk
  out   = Relu(conv + beta)          (ACT epilogue, per-partition bias;
                                      BN scale folded into bank/bias on host;
                                      fp16 output, host upconverts)

Layout: x is host-transposed to channel-major [C, H, W], zero-padded to
[C, 66, 66], and cast to fp16 so all 9 conv taps are plain access-pattern
offsets; output is produced channel-major [F, H*W] fp16 and host-transposed
back to NHWC. Expert bank is BN-folded, fp16, tap-group-major, replicated
per core.
"""

import os
import sys

if "/opt/trn_rl_repo" not in sys.path:
    sys.path.insert(0, "/opt/trn_rl_repo")
# The kernel executes through the axon PJRT backend; make sure jax can see it
# if the caller's environment doesn't pin a platform.
if not os.environ.get("JAX_PLATFORMS"):
    os.environ["JAX_PLATFORMS"] = "axon"

import numpy as np

import concourse.bacc as bacc
import concourse.tile as tile
from concourse import mybir
from concourse.bass_utils import run_bass_kernel_spmd
from concourse.tile_rust import add_dep_helper


def _ensure_ntff_hook():
    """run_bass_kernel_spmd(trace=True) under axon needs antenv.axon_hooks,
    which this image's antenv package lacks. Register an equivalent module
    (ctypes into libaxon_pjrt.so) so profiled runs work."""
    try:
        from antenv import axon_hooks  # noqa: F401
        return
    except ImportError:
        pass
    import contextlib
    import ctypes
    import os
    import types

    so_path = os.environ.get("AXON_PJRT_SO", "/opt/axon/libaxon_pjrt.so")
    mod = types.ModuleType("antenv.axon_hooks")
    state = {"hook": None}

    def _make_hook():
        if not os.path.exists(so_path):
            return None
        lib = ctypes.CDLL(so_path)
        if not hasattr(lib, "axon_start_nrt_profile"):
            return None
        lib.axon_start_nrt_profile.argtypes = [
            ctypes.POINTER(ctypes.c_int64), ctypes.c_size_t]
        lib.axon_start_nrt_profile.restype = ctypes.c_int64
        lib.axon_stop_nrt_profile.argtypes = [ctypes.c_char_p]
        lib.axon_stop_nrt_profile.restype = ctypes.c_int64

        @contextlib.contextmanager
        def _hook(output_dir, device_ids):
            import jax
            jax.devices()
            if device_ids:
                ids = (ctypes.c_int64 * len(device_ids))(*device_ids)
                rc = lib.axon_start_nrt_profile(ids, len(device_ids))
            else:
                rc = lib.axon_start_nrt_profile(None, 0)
            if rc != 0:
                raise RuntimeError(f"axon_start_nrt_profile rc={rc}")
            try:
                yield
            finally:
                n = lib.axon_stop_nrt_profile(str(output_dir).encode())
                if n < 0:
                    raise RuntimeError(f"axon_stop_nrt_profile rc={n}")

        return _hook

    def get_axon_ntff_profile_hook():
        if state["hook"] is None:
            state["hook"] = _make_hook()
        return state["hook"]

    def set_axon_ntff_profile_hook(hook):
        state["hook"] = hook

    mod.get_axon_ntff_profile_hook = get_axon_ntff_profile_hook
    mod.set_axon_ntff_profile_hook = set_axon_ntff_profile_hook
    sys.modules["antenv.axon_hooks"] = mod
    try:
        import antenv
        antenv.axon_hooks = mod
    except ImportError:
        pass


F32 = mybir.dt.float32
F16 = mybir.dt.float16
AF = mybir.ActivationFunctionType
ALU = mybir.AluOpType

B, H, W, C = 32, 64, 64, 128
NCORES = 8
BPC = B // NCORES  # samples per core
HP, WP = H + 2, W + 2  # zero-padded
NPAD = HP * WP  # 4356
NPOS = H * W  # 4096
K = 4  # experts
NF = 128  # output filters
TAPS = 9
ROWS_PER_CHUNK = 8  # 8 image rows * 64 cols = 512 positions per PSUM chunk
NCHUNK = H // ROWS_PER_CHUNK
HALF = NPAD // 2
NPIECE = 6  # sample-0 startup DMA pieces
PIECE = NPAD // NPIECE  # 726
GROUPS = 3  # mixing tap-groups
GW = 3 * NF  # 384: wm cols per group
WGK = K * GW  # 1536: wk cols per group (group-major bank layout)
# consts packed in one [128, 137] tile: routing-critical red/attk in the
# first 8 cols, beta-path biasw/c1 after (one mid-priority DMA on the
# scalar ring — a separate tiny "hot" DMA fragments into 128 32-byte
# packets and clogs the ring)
HOT_COLS = 8
CONST_COLS = 137

# tunables: warm-up matmul counts (keep PE busy/clock-ramped through startup)
WARM1 = 9    # 512-col fp16 warm-ups before chain-0 tiny matmuls
WARM2 = 2    # 256-col warm-ups between chain-0 steps (covers engine hops)
WARM3 = 8    # 512-col warm-ups covering the mixing-group-0 window


class _Consts:
    """AP views into the packed constant SBUF tiles."""

    def __init__(self, consts, ones1):
        self.red = consts[:, 0:4]          # reduction_kernel / 4096  [128, 4]
        self.attk = consts[0:4, 4:8]       # attention_kernel / 30    [4, 4]
        self.biasw = consts[0:4, 8:136]    # bias * inv               [4, 128]
        self.c1 = consts[:, 136:137]       # bn_bias - bn_mean*inv    [128, 1]
        self.ones1 = ones1                 # ones (memset on device)  [1, 128]


def _pack_consts(red, attk, biasw, c1):
    consts = np.zeros((128, CONST_COLS), dtype=np.float32)
    consts[:, 0:4] = red
    consts[0:4, 4:8] = attk
    consts[0:4, 8:136] = biasw
    consts[:, 136] = c1
    return consts


def _emit_pool(nc, b, sb, xt_sb, trash, act_only=False):
    """Pool half-reduces for samples >= 1 (ACT + DVE). act_only puts both
    halves on the scalar engine — used for the last samples so their
    reduces take no DVE time away from the mixing chains."""
    pq = [sb.tile([C, 1], F32, tag="poolh", name=f"pool{b}h{i}")
          for i in range(2)]
    ia = nc.scalar.activation(trash[:, :HALF], xt_sb[:, :HALF], AF.Identity,
                              accum_out=pq[0][:])
    if act_only:
        ib = nc.scalar.activation(trash[:, :HALF], xt_sb[:, HALF:],
                                  AF.Identity, accum_out=pq[1][:])
    else:
        ib = nc.vector.tensor_reduce(pq[1][:], xt_sb[:, HALF:],
                                     axis=mybir.AxisListType.X, op=ALU.add)
    return {"pool_a": ia, "pool_b": ib, "pq": [q[:] for q in pq],
            "act_only": act_only}


def _emit_chain(nc, b, sb, ps, cc, wk_sb, wm_sb, beta_sb, invs_sb, pool,
                warm=None):
    """Routing chain for sample b: pool partials -> pr(PE) -> relu(ACT) ->
    lg_row(PE) -> exp(ACT, softmax denominator free via accum_out) -> att
    broadcast(PE) -> copy(DVE) -> mixing MACs(DVE, 3 tap-groups).

    The softmax is left UNNORMALIZED — mixing uses raw exp weights and the
    1/sum lands in the epilogue's per-partition activation scale (invs_sb),
    along with the matching bias correction (beta_sb). That normalization
    branch runs off the critical path, in parallel with the mixing.

    wk_sb(g, k) returns the bank slice for tap-group g of expert k; mixing
    is emitted group-by-group so conv taps 3g..3g+2 unblock as soon as
    group g lands in wm_sb.
    """
    pq = pool["pq"]

    # pool_red.T = (R/4096).T @ sum(pq) via accumulating matmuls
    pr_ps = ps.tile([K, 1], F32, tag="tiny")
    n = len(pq)
    for i in range(n):
        nc.tensor.matmul(pr_ps[:], cc.red, pq[i], start=(i == 0),
                         stop=(i == n - 1))
    if warm:
        warm()
    prelu_sb = sb.tile([K, 1], F32, tag="prelu")
    nc.scalar.activation(prelu_sb[:], pr_ps[:], AF.Relu)

    # logits as a ROW: lg_row = pool_red @ (A/30)  -> [1, 4]
    lgr_ps = ps.tile([1, K], F32, tag="tiny")
    nc.tensor.matmul(lgr_ps[:], prelu_sb[:], cc.attk, start=True, stop=True)
    if warm:
        warm()
    # e_row = exp(lg_row); the softmax denominator comes free via accum_out
    er_sb = sb.tile([1, K], F32, tag="erow")
    s_sb = sb.tile([1, 1], F32, tag="ssum")
    nc.scalar.activation(er_sb[:], lgr_ps[:], AF.Exp, accum_out=s_sb[:])
    # broadcast raw exp weights to all 128 partitions
    ab_ps = ps.tile([C, K], F32, tag="tiny")
    nc.tensor.matmul(ab_ps[:], cc.ones1, er_sb[:], start=True, stop=True)
    ab_sb = sb.tile([C, K], F32, tag="abc")
    nc.vector.tensor_copy(ab_sb[:], ab_ps[:])

    # Mix expert bank with UNNORMALIZED weights, one tap-group at a time:
    # wm[:, g] = sum_k e[k] * wk[g, k]
    last = None
    for g in range(GROUPS):
        dst = wm_sb[:, g * GW:(g + 1) * GW]
        nc.vector.tensor_scalar_mul(dst, wk_sb(g, 0), ab_sb[:, 0:1])
        for k in range(1, K):
            last = nc.vector.scalar_tensor_tensor(
                dst, wk_sb(g, k), ab_sb[:, k:k + 1], dst,
                op0=ALU.mult, op1=ALU.add)

    # Off-critical-path normalization: invs = broadcast(1/s) for the
    # epilogue scale; beta = (biasw.T @ e) * invs + c1 for the epilogue bias.
    rec_sb = sb.tile([1, 1], F32, tag="rec")
    nc.vector.reciprocal(rec_sb[:], s_sb[:])
    invs_ps = ps.tile([C, 1], F32, tag="tiny")
    nc.tensor.matmul(invs_ps[:], cc.ones1, rec_sb[:], start=True, stop=True)
    nc.vector.tensor_copy(invs_sb[:], invs_ps[:])
    lgc_ps = ps.tile([K, 1], F32, tag="tiny")
    nc.tensor.matmul(lgc_ps[:], cc.attk, prelu_sb[:], start=True, stop=True)
    ec_sb = sb.tile([K, 1], F32, tag="ecol")
    nc.scalar.activation(ec_sb[:], lgc_ps[:], AF.Exp)
    bm_ps = ps.tile([NF, 1], F32, tag="tiny")
    nc.tensor.matmul(bm_ps[:], cc.biasw, ec_sb[:], start=True, stop=True)
    nc.vector.tensor_scalar(beta_sb[:], bm_ps[:], invs_sb[:], cc.c1,
                            op0=ALU.mult, op1=ALU.add)
    pool["mix_last"] = last
    return pool


def _emit_conv_chunks(nc, b, convps, xt_sb, wm_sb, beta_sb, invs_sb, y_sb,
                      y_dram, t_lo, t_hi, last_sample=False):
    """9-tap conv chunks [t_lo, t_hi) as shifted fp16 matmuls + fused
    BN/bias/relu epilogue; fp16 output DMA'd out in pieces (sync + vector
    sync + gpsimd queues, keeping the scalar queue free for epilogues)."""
    xv = xt_sb.rearrange("p (h w) -> p h w", w=WP)
    for t in range(t_lo, t_hi):
        pc = convps.tile([NF, ROWS_PER_CHUNK * W], F32, tag="conv")
        for tap in range(TAPS):
            dy, dx = tap // 3, tap % 3
            r0 = ROWS_PER_CHUNK * t + dy
            rhs = xv[:, r0:r0 + ROWS_PER_CHUNK, dx:dx + W]
            nc.tensor.matmul(pc[:], wm_sb[:, NF * tap:NF * (tap + 1)], rhs,
                             start=(tap == 0), stop=(tap == TAPS - 1))
        if last_sample and t == NCHUNK - 1:
            # split the final epilogue so the tail DMA starts sooner
            nc.scalar.activation(y_sb[:, 3584:3840], pc[:, 0:256], AF.Relu,
                                 bias=beta_sb[:], scale=invs_sb[:])
            nc.gpsimd.dma_start(y_dram[b][:, 3584:3840], y_sb[:, 3584:3840])
            nc.scalar.activation(y_sb[:, 3840:4096], pc[:, 256:512], AF.Relu,
                                 bias=beta_sb[:], scale=invs_sb[:])
            nc.sync.dma_start(y_dram[b][:, 3840:], y_sb[:, 3840:])
            continue
        nc.scalar.activation(y_sb[:, 512 * t:512 * (t + 1)], pc[:], AF.Relu,
                             bias=beta_sb[:], scale=invs_sb[:])
        if t == 3:
            nc.sync.dma_start(y_dram[b][:, :2048], y_sb[:, :2048])
        elif t == 5:
            nc.gpsimd.dma_start(y_dram[b][:, 2048:3072], y_sb[:, 2048:3072])
        elif t == 6:
            nc.sync.dma_start(y_dram[b][:, 3072:3584], y_sb[:, 3072:3584])
        elif t == 7:
            nc.gpsimd.dma_start(y_dram[b][:, 3584:3840], y_sb[:, 3584:3840])
            nc.sync.dma_start(y_dram[b][:, 3840:], y_sb[:, 3840:])


def _emit_conv_sample0(nc, convps, xt_sb, wm_sb, beta_sb, invs_sb, y_sb,
                       y_dram):
    """Sample-0 conv with the first 4 chunks pipelined against the 3-group
    mixing: pass g runs taps 3g..3g+2 over chunks 0-3, so the conv starts
    as soon as mixing group 0 lands, while groups 1-2 are still mixing."""
    xv = xt_sb.rearrange("p (h w) -> p h w", w=WP)
    NPIPE = 4
    pcs = [convps.tile([NF, ROWS_PER_CHUNK * W], F32, tag="conv",
                       name=f"c0p{c}") for c in range(NPIPE)]
    for g in range(GROUPS):
        for c in range(NPIPE):
            for tap in range(3 * g, 3 * g + 3):
                dy, dx = tap // 3, tap % 3
                r0 = ROWS_PER_CHUNK * c + dy
                rhs = xv[:, r0:r0 + ROWS_PER_CHUNK, dx:dx + W]
                nc.tensor.matmul(pcs[c][:],
                                 wm_sb[:, NF * tap:NF * (tap + 1)], rhs,
                                 start=(tap == 0), stop=(tap == TAPS - 1))
    for c in range(NPIPE):
        nc.scalar.activation(y_sb[:, 512 * c:512 * (c + 1)], pcs[c][:],
                             AF.Relu, bias=beta_sb[:], scale=invs_sb[:])
        if c == 3:
            nc.sync.dma_start(y_dram[0][:, :2048], y_sb[:, :2048])


def _build_program():
    nc = bacc.Bacc("TRN2", target_bir_lowering=False, debug=False,
                   num_devices=NCORES)
    xt = nc.dram_tensor("xt", [BPC, C, NPAD], F16, kind="ExternalInput").ap()
    wk = nc.dram_tensor("wk", [C, GROUPS * WGK], F16,
                        kind="ExternalInput").ap()
    constd = nc.dram_tensor("consts", [128, CONST_COLS], F32,
                            kind="ExternalInput").ap()
    y = nc.dram_tensor("y", [BPC, NF, NPOS], F16, kind="ExternalOutput").ap()

    with tile.TileContext(nc) as tc:
        with (
            tc.tile_pool(name="const", bufs=1) as cpool,
            tc.tile_pool(name="xt", bufs=BPC) as xpool,
            tc.tile_pool(name="wm", bufs=BPC) as wmpool,
            tc.tile_pool(name="work", bufs=4) as sb,
            tc.tile_pool(name="ystage", bufs=2) as ypool,
            tc.tile_pool(name="convps", bufs=5, space="PSUM") as convps,
            tc.tile_pool(name="tinyps", bufs=2, space="PSUM") as ps,
        ):
            xt_sb = [xpool.tile([C, NPAD], F16, tag="xt", name=f"xt{b}")
                     for b in range(BPC)]
            # On-device constants: ones row + zeroed warm-up matmul source
            # (available immediately, no DMA).
            ones1_sb = cpool.tile([1, C], F32, tag="ones1")
            nc.gpsimd.memset(ones1_sb[:], 1.0)
            warm_src = cpool.tile([C, 512], F16, tag="warmsrc")
            nc.gpsimd.memset(warm_src[:], 0.0)

            consts = cpool.tile([128, CONST_COLS], F32)
            wk_all = cpool.tile([C, GROUPS * WGK], F16)

            # Startup loads across the three DMA rings (sync / scalar /
            # gpsimd queues; each ring sustains only ~134 GB/s, so ring
            # assignment decides arrival time). Sample 0's input goes in 6
            # pieces round-robin so pool partial-reduces chase the DMA; the
            # bank follows in tap-group order (group-major layout) so mixing
            # group g never waits on groups g+1. The gpsimd ring stops after
            # its two x0 pieces so the gpsimd ENGINE is free for its pool
            # partials; wk halves ride sync + scalar.
            qs = [nc.sync, nc.scalar, nc.gpsimd]
            for i in range(NPIECE):
                q = qs[i % 3]
                q.dma_start(xt_sb[0][:, i * PIECE:(i + 1) * PIECE],
                            xt[0][:, i * PIECE:(i + 1) * PIECE])
            HG = WGK // 2  # 768: half-group piece
            nc.scalar.dma_start(consts[:], constd)
            for g in range(GROUPS):
                base = g * WGK
                nc.sync.dma_start(wk_all[:, base:base + HG],
                                  wk[:, base:base + HG])
                nc.scalar.dma_start(wk_all[:, base + HG:base + WGK],
                                    wk[:, base + HG:base + WGK])
            cc = _Consts(consts[:], ones1_sb[:])

            def wk_sb(g, k):
                base = g * WGK + k * GW
                return wk_all[:, base:base + GW]

            # Pre-load the ACT spline table set (relu+exp share one set).
            warm_sb = cpool.tile([1, 1], F32, tag="warm")
            nc.scalar.activation(warm_sb[:], ones1_sb[:, 0:1], AF.Exp)

            trash = cpool.tile([C, NPAD], F16, tag="trash")

            wm_sb = [wmpool.tile([C, TAPS * NF], F16, tag="wm",
                                 name=f"wm{b}") for b in range(BPC)]
            beta_sb = [sb.tile([NF, 1], F32, tag="beta", name=f"beta{b}")
                       for b in range(BPC)]
            invs_sb = [sb.tile([NF, 1], F32, tag="invs", name=f"invs{b}")
                       for b in range(BPC)]
            y_sb = [ypool.tile([NF, NPOS], F16, tag="ystage", name=f"yst{b}")
                    for b in range(BPC)]

            # PE warm-up: fine-grained fp16 matmuls on the memset source so
            # the array stays busy (HAM at full clock) through the startup
            # window.
            warm_ps = ps.tile([NF, 512], F32, tag="warmps", bufs=1)

            def pe_warm(n, cols=256):
                for _ in range(n):
                    nc.tensor.matmul(warm_ps[:, :cols], warm_src[:, 0:NF],
                                     warm_src[:, 0:cols], start=True,
                                     stop=True)

            def emit_next_xt(bn, prev):
                # Sample bn's input on the GPSIMD + scalar rings, gated on
                # sample bn-1's input being fully resident (its last pool
                # quarter-reduces) so transfers don't fight for HBM early.
                da = nc.gpsimd.dma_start(xt_sb[bn][:, :HALF],
                                         xt[bn][:, :HALF])
                db = nc.scalar.dma_start(xt_sb[bn][:, HALF:],
                                         xt[bn][:, HALF:])
                add_dep_helper(da.ins, prev["pool_a"].ins,
                               reason="stagger input DMA bandwidth")
                add_dep_helper(db.ins, prev["pool_b"].ins,
                               reason="stagger input DMA bandwidth")

            pe_warm(WARM1, cols=512)

            # Sample 0: piecewise pool partials chase the 6 DMA pieces
            # (ACT for even pieces, DVE for odd); no combine step — the pr
            # matmul accumulates all 6 partials directly, ordered by
            # expected completion so the PE consumes them as they finish.
            pq6 = sb.tile([C, NPIECE], F32, tag="pq6")
            pool0_ins = []
            for i in range(NPIECE):
                piece = xt_sb[0][:, i * PIECE:(i + 1) * PIECE]
                if i % 2 == 0:
                    ins = nc.scalar.activation(trash[:, :PIECE], piece,
                                               AF.Identity,
                                               accum_out=pq6[:, i:i + 1])
                else:
                    ins = nc.vector.tensor_reduce(pq6[:, i:i + 1], piece,
                                                  axis=mybir.AxisListType.X,
                                                  op=ALU.add)
                pool0_ins.append(ins)
            chains = [None] * BPC
            chains[0] = {"pool_a": pool0_ins[-2], "pool_b": pool0_ins[-1],
                         "pq": [pq6[:, i:i + 1]
                                for i in (1, 0, 3, 2, 5, 4)]}
            _emit_chain(nc, 0, sb, ps, cc, wk_sb, wm_sb[0], beta_sb[0],
                        invs_sb[0], chains[0],
                        warm=lambda: pe_warm(WARM2))
            emit_next_xt(1, chains[0])
            pe_warm(WARM3, cols=512)

            # Sample 0 conv: first 4 chunks pipelined against mixing groups.
            _emit_conv_sample0(nc, convps, xt_sb[0][:], wm_sb[0], beta_sb[0],
                               invs_sb[0], y_sb[0], y)

            # Per sample b: the NEXT sample's pool reduces go ahead of
            # conv_b's chunks (so the ACT-side reduce precedes conv_b's
            # epilogues in the scalar-engine queue); the rest of its routing
            # chain is emitted between chunk groups.
            for b in range(BPC):
                if b + 1 < BPC:
                    chains[b + 1] = _emit_pool(nc, b + 1, sb,
                                               xt_sb[b + 1][:], trash,
                                               act_only=(b + 1 >= BPC - 2))
                    if not chains[b + 1]["act_only"]:
                        add_dep_helper(chains[b + 1]["pool_b"].ins,
                                       chains[b]["mix_last"].ins,
                                       reason="keep DVE reduce after prev mix")
                if b == 0:
                    _emit_conv_chunks(nc, b, convps, xt_sb[b][:], wm_sb[b],
                                      beta_sb[b], invs_sb[b], y_sb[b], y,
                                      4, 5)
                else:
                    _emit_conv_chunks(nc, b, convps, xt_sb[b][:], wm_sb[b],
                                      beta_sb[b], invs_sb[b], y_sb[b], y,
                                      0, 3)
                if b + 1 < BPC:
                    _emit_chain(nc, b + 1, sb, ps, cc, wk_sb, wm_sb[b + 1],
                                beta_sb[b + 1], invs_sb[b + 1],
                                chains[b + 1])
                    if b + 2 < BPC:
                        emit_next_xt(b + 2, chains[b + 1])
                _emit_conv_chunks(nc, b, convps, xt_sb[b][:], wm_sb[b],
                                  beta_sb[b], invs_sb[b], y_sb[b], y,
                                  5 if b == 0 else 3, NCHUNK,
                                  last_sample=(b == BPC - 1))

    nc.compile()
    return nc


_PROGRAM = None


def _get_program():
    global _PROGRAM
    if _PROGRAM is None:
        _PROGRAM = _build_program()
    return _PROGRAM


def _prepare_host_inputs(x, reduction_kernel, attention_kernel, conv_kernels,
                         bias, bn_scale, bn_bias, bn_mean, bn_var):
    f = np.float32
    # Channel-major zero-padded fp16 input: [B, C, 66, 66]
    xt = np.zeros((B, C, HP, WP), dtype=np.float16)
    xt[:, :, 1:H + 1, 1:W + 1] = x.transpose(0, 3, 1, 2)
    xt = xt.reshape(B, C, NPAD)

    inv = (bn_scale / np.sqrt(bn_var + np.float32(1e-5))).astype(f)
    # Expert bank fp16, BN scale folded into F, tap-GROUP-major layout:
    # [C, g, k, 3, F] so each mixing group is one contiguous block.
    wkh = (conv_kernels.transpose(0, 3, 1, 2, 4) * inv).astype(f)  # [K,C,3,3,F]
    wkh = wkh.reshape(K, C, GROUPS, 3 * NF).transpose(1, 2, 0, 3)  # [C,g,K,384]
    wkh = np.ascontiguousarray(wkh.reshape(C, GROUPS * WGK), dtype=np.float16)

    consts = _pack_consts(
        red=(reduction_kernel / np.float32(NPOS)).astype(f),
        attk=(attention_kernel / np.float32(30.0)).astype(f),
        biasw=(bias * inv).astype(f),
        c1=(bn_bias - bn_mean * inv).astype(f),
    )

    in_maps = []
    for cix in range(NCORES):
        in_maps.append({
            "xt": np.ascontiguousarray(xt[cix * BPC:(cix + 1) * BPC]),
            "wk": wkh,
            "consts": consts,
        })
    return in_maps


def kernel(x, reduction_kernel, attention_kernel, conv_kernels, bias, bn_scale,
           bn_bias, bn_mean, bn_var, _trace=False):
    nc = _get_program()
    in_maps = _prepare_host_inputs(
        np.asarray(x, dtype=np.float32), np.asarray(reduction_kernel, np.float32),
        np.asarray(attention_kernel, np.float32),
        np.asarray(conv_kernels, np.float32), np.asarray(bias, np.float32),
        np.asarray(bn_scale, np.float32), np.asarray(bn_bias, np.float32),
        np.asarray(bn_mean, np.float32), np.asarray(bn_var, np.float32))
    if _trace:
        _ensure_ntff_hook()
    res = run_bass_kernel_spmd(nc, in_maps, core_ids=list(range(NCORES)),
                               trace=_trace)
    yt = np.concatenate([res.results[cix]["y"] for cix in range(NCORES)],
                        axis=0)  # [B, F, 4096] fp16
    out = yt.astype(np.float32).reshape(B, NF, H, W).transpose(0, 2, 3, 1)
    out = np.ascontiguousarray(out, dtype=np.float32)
    if _trace:
        return out, res
    return out
